# revision 2
# baseline (speedup 1.0000x reference)
"""2D Haar DWT (single level) on Trainium2, 8 NeuronCores, pure data parallel.

Math: with Haar filters + symmetric pad + odd-phase downsample, the DWT
reduces to per-2x2-block butterflies over the input image x:
  ll = 0.5*(x00 + x01 + x10 + x11)   (top-left quadrant of output)
  lh = 0.5*(x00 + x01 - x10 - x11)   (bottom-left)
  hl = 0.5*(x00 - x01 + x10 - x11)   (top-right)
  hh = 0.5*(x00 - x01 - x10 + x11)   (bottom-right)

Layout per image (512x512 f32): one contiguous 1 MiB DMA into X[128, 2048]
(partition p holds rows 4p..4p+3). ACT scales by 0.5, DVE does the width
butterflies (stride-2 reads), GPSIMD does the height butterflies straight
into the assembled output tile Y[128, 2048], one 1 MiB DMA out.
Y[p, c*1024 + q*512 + w] = out[c*256 + 2p + q, w].
"""

import numpy as np

import concourse.mybir as mybir
from concourse import bacc, tile
from concourse.bass_utils import run_bass_kernel_spmd

N_CORES = 8
BATCH = 64
B_PER = BATCH // N_CORES  # 8 images per core
H = W = 512

_nc_cache = None


def build_bass():
    f32 = mybir.dt.float32
    nc = bacc.Bacc(
        "TRN2", target_bir_lowering=False, debug=False, num_devices=N_CORES
    )
    inp = nc.dram_tensor("inputs", [B_PER, H, W], f32, kind="ExternalInput").ap()
    out = nc.dram_tensor("out", [B_PER, H, W], f32, kind="ExternalOutput").ap()

    with tile.TileContext(nc) as tc:
        with tc.tile_pool(name="p", bufs=3) as pool:
            for i in range(B_PER):
                X = pool.tile([128, 2048], f32, tag="X")
                nc.sync.dma_start(
                    out=X[:], in_=inp[i].rearrange("(p r) w -> p (r w)", p=128)
                )
                X2 = pool.tile([128, 2048], f32, tag="X2")
                nc.scalar.mul(X2[:], X[:], 0.5)

                # width pass: T[:, r*256:...] = row sums, T[:, 1024+r*256:...] = row diffs
                T = pool.tile([128, 2048], f32, tag="T")
                for r in range(4):
                    e = X2[:, r * 512 : (r + 1) * 512 : 2]
                    o = X2[:, r * 512 + 1 : (r + 1) * 512 : 2]
                    nc.vector.tensor_add(
                        out=T[:, r * 256 : (r + 1) * 256], in0=e, in1=o
                    )
                    nc.vector.tensor_sub(
                        out=T[:, 1024 + r * 256 : 1024 + (r + 1) * 256], in0=e, in1=o
                    )

                # height pass into assembled output tile
                Y = pool.tile([128, 2048], f32, tag="Y")
                for q in range(2):
                    ts0 = T[:, (2 * q) * 256 : (2 * q + 1) * 256]
                    ts1 = T[:, (2 * q + 1) * 256 : (2 * q + 2) * 256]
                    td0 = T[:, 1024 + (2 * q) * 256 : 1024 + (2 * q + 1) * 256]
                    td1 = T[:, 1024 + (2 * q + 1) * 256 : 1024 + (2 * q + 2) * 256]
                    base = q * 512
                    nc.gpsimd.tensor_add(out=Y[:, base : base + 256], in0=ts0, in1=ts1)
                    nc.gpsimd.tensor_add(
                        out=Y[:, base + 256 : base + 512], in0=td0, in1=td1
                    )
                    nc.gpsimd.tensor_sub(
                        out=Y[:, 1024 + base : 1024 + base + 256], in0=ts0, in1=ts1
                    )
                    nc.gpsimd.tensor_sub(
                        out=Y[:, 1024 + base + 256 : 1024 + base + 512],
                        in0=td0,
                        in1=td1,
                    )

                nc.sync.dma_start(
                    out=out[i].rearrange("(c p q) w -> p c q w", c=2, q=2),
                    in_=Y[:],
                )

    nc.compile()
    return nc


def kernel(**inputs):
    global _nc_cache
    x = np.ascontiguousarray(
        np.asarray(inputs["inputs"], dtype=np.float32).reshape(BATCH, H, W)
    )
    if _nc_cache is None:
        _nc_cache = build_bass()
    nc = _nc_cache
    in_maps = [
        {"inputs": x[i * B_PER : (i + 1) * B_PER]} for i in range(N_CORES)
    ]
    res = run_bass_kernel_spmd(nc, in_maps, core_ids=list(range(N_CORES))).results
    out = np.concatenate([res[i]["out"] for i in range(N_CORES)], axis=0)
    return out.reshape(BATCH, H, W, 1)


# revision 3
# speedup vs baseline: 1.2300x; 1.2300x over previous
"""2D Haar DWT (single level) on Trainium2, 8 NeuronCores, pure data parallel.

Math: with Haar filters + symmetric pad + odd-phase downsample, the DWT
reduces to per-2x2-block butterflies over the input image x:
  ll = 0.5*(x00 + x01 + x10 + x11)   (top-left quadrant of output)
  lh = 0.5*(x00 + x01 - x10 - x11)   (bottom-left)
  hl = 0.5*(x00 - x01 + x10 - x11)   (top-right)
  hh = 0.5*(x00 - x01 - x10 + x11)   (bottom-right)

Layout per image (512x512 f32): one contiguous 1 MiB DMA into X[128, 2048]
(partition p holds rows 4p..4p+3).  ACT scales by 0.5.  Width-pass pair
SUMS via a single DVE tensor_reduce with sequential reads (DVE pays ~3x
for strided reads, so avoid them); width-pass pair DIFFS on GpSimd with
strided reads (software engine, stride-insensitive).  Height pass =
contiguous adds/subs split DVE/GpSimd straight into the assembled output
tile Y[128, 2048]; one 1 MiB DMA out.
Y[p, c*1024 + q*512 + w] = out[c*256 + 2p + q, w].
"""

import numpy as np

import concourse.mybir as mybir
from concourse import bacc, tile
from concourse.bass_utils import run_bass_kernel_spmd

N_CORES = 8
BATCH = 64
B_PER = BATCH // N_CORES  # 8 images per core
H = W = 512

_nc_cache = None


def build_bass():
    f32 = mybir.dt.float32
    nc = bacc.Bacc(
        "TRN2", target_bir_lowering=False, debug=False, num_devices=N_CORES
    )
    inp = nc.dram_tensor("inputs", [B_PER, H, W], f32, kind="ExternalInput").ap()
    out = nc.dram_tensor("out", [B_PER, H, W], f32, kind="ExternalOutput").ap()

    with tile.TileContext(nc) as tc:
        with tc.tile_pool(name="p", bufs=3) as pool:
            for i in range(B_PER):
                X = pool.tile([128, 2048], f32, tag="X")
                nc.sync.dma_start(
                    out=X[:], in_=inp[i].rearrange("(p r) w -> p (r w)", p=128)
                )
                X2 = pool.tile([128, 2048], f32, tag="X2")
                nc.scalar.mul(X2[:], X[:], 0.5)

                # width pass: T[:, 0:1024] = pair sums (r-blocks of 256),
                #             T[:, 1024:2048] = pair diffs
                T = pool.tile([128, 2048], f32, tag="T")
                # sums: one reduce, fully sequential reads on DVE
                nc.vector.tensor_reduce(
                    out=T[:, 0:1024],
                    in_=X2[:].rearrange("p (r k t) -> p r k t", r=4, t=2),
                    axis=mybir.AxisListType.X,
                    op=mybir.AluOpType.add,
                )
                # diffs: strided reads on GpSimd (stride-insensitive)
                for r in range(4):
                    nc.gpsimd.tensor_sub(
                        out=T[:, 1024 + r * 256 : 1024 + (r + 1) * 256],
                        in0=X2[:, r * 512 : (r + 1) * 512 : 2],
                        in1=X2[:, r * 512 + 1 : (r + 1) * 512 : 2],
                    )

                # height pass into assembled output tile
                # Y[:, q*512+0:256]=ll_q  [q*512+256:512]=hl_q
                # Y[:, 1024+q*512+0:256]=lh_q  [...+256:512]=hh_q
                Y = pool.tile([128, 2048], f32, tag="Y")
                for q in range(2):
                    ts0 = T[:, (2 * q) * 256 : (2 * q + 1) * 256]
                    ts1 = T[:, (2 * q + 1) * 256 : (2 * q + 2) * 256]
                    td0 = T[:, 1024 + (2 * q) * 256 : 1024 + (2 * q + 1) * 256]
                    td1 = T[:, 1024 + (2 * q + 1) * 256 : 1024 + (2 * q + 2) * 256]
                    base = q * 512
                    # DVE: ll, lh, hl ; GpSimd: hh  (6/2 split)
                    nc.vector.tensor_add(out=Y[:, base : base + 256], in0=ts0, in1=ts1)
                    nc.vector.tensor_sub(
                        out=Y[:, 1024 + base : 1024 + base + 256], in0=ts0, in1=ts1
                    )
                    nc.vector.tensor_add(
                        out=Y[:, base + 256 : base + 512], in0=td0, in1=td1
                    )
                    nc.gpsimd.tensor_sub(
                        out=Y[:, 1024 + base + 256 : 1024 + base + 512],
                        in0=td0,
                        in1=td1,
                    )

                nc.sync.dma_start(
                    out=out[i].rearrange("(c p q) w -> p c q w", c=2, q=2),
                    in_=Y[:],
                )

    nc.compile()
    return nc


def kernel(**inputs):
    global _nc_cache
    x = np.ascontiguousarray(
        np.asarray(inputs["inputs"], dtype=np.float32).reshape(BATCH, H, W)
    )
    if _nc_cache is None:
        _nc_cache = build_bass()
    nc = _nc_cache
    in_maps = [
        {"inputs": x[i * B_PER : (i + 1) * B_PER]} for i in range(N_CORES)
    ]
    res = run_bass_kernel_spmd(nc, in_maps, core_ids=list(range(N_CORES))).results
    out = np.concatenate([res[i]["out"] for i in range(N_CORES)], axis=0)
    return out.reshape(BATCH, H, W, 1)
